# revision 38
# baseline (speedup 1.0000x reference)
"""Trainium2 Bass kernel for nn_MultiHeadAttention_51986284150861 (fp8 rev).

Full transformer block (MHA + LN1 + FFN; LN2 applied exactly on host) with
the reference's faithful torch-.view head split: head a attends over 2048
positions (j, t') where position (j, t') draws features 64j..64j+63 of
token 128a + t'.  Per core: batch b = c//4, 512 tokens = 4 head windows.
Zero collectives.

Pipelined emission so the ready-heap tile scheduler overlaps the ACT-bound
softmax of pair P+1 with the PE-bound FFN of pair P:
  attn(w0) attn(w1) | LN1(0) FFN1(0) gelu(0) | attn(w2) ~ FFN2(0) ~ attn(w3)
  | LN1(1) FFN1(1) gelu(1) FFN2(1)
PSUM rings: strips/wops [128,1024]x2 (4 banks), fps/stats [128,256]x2
(2 banks), f2/pav [128,512]x2 (2 banks).  FFN2 runs as 4 fo-pair passes
(W2 streamed once per pair).  FFN1 pre-gelu drains to SBUF (DVE/ACT
split), gelu batched as [128,1024] tiles to bound ACT table switches.
Softmax division: DVE recip -> Pool partition_broadcast -> DVE mul.
"""

import os
import sys

sys.path.insert(0, "/opt/trn_rl_repo")

from contextlib import ExitStack

import numpy as np

import concourse.bass as bass
import concourse.tile as tile
from concourse import bacc, mybir

F32 = mybir.dt.float32
F32R = mybir.dt.float32r
BF16 = mybir.dt.bfloat16
FP8 = mybir.dt.float8e4
DR = mybir.MatmulPerfMode.DoubleRow
AF = mybir.ActivationFunctionType
OP = mybir.AluOpType

B, T, H, NH, D = 2, 2048, 1024, 16, 64
EPS = 1e-5
NCORES = 8
GROUP = NCORES // B
TQ = T // GROUP              # 512 tokens/core
NW = 4                       # head windows per core
JB = 16                      # feature blocks
SW = 32.0                    # fp8 weight scale
SQ = 1024.0                  # q residual prescale (= SW*SW)
EXPSC = 1.0 / (8.0 * SW * SW)
MASK_NEG = -80.0
EPS1 = EPS * SQ * SQ         # LN1 eps (x is 1024-scaled)

# (w, jk, half) triples whose softmax goes DVE-affine + Pool-square instead
# of ACT exp.
N_OFF = int(os.environ.get("MHA_N_OFF", "0"))
OFF_JKS = [1, 3, 5, 7, 9, 11, 2, 6, 10, 14]
OFF_WINS = int(os.environ.get("MHA_OFF_WINS", "0"))
OFFLOAD = {(w, jk, half) for w in range(OFF_WINS) for half in range(2)
           for jk in OFF_JKS[:N_OFF]}
# FFN1 pre-gelu drains: which ot indices drain via ACT Copy (rest on DVE)
ACT_Z = int(os.environ.get("MHA_ACT_Z", "0"))  # of every 4


def _offload(w, jk, half):
    return (w, jk, half) in OFFLOAD


def build_program():
    nc = bacc.Bacc("TRN2", target_bir_lowering=False, debug=False)

    def din(name, shape, dt=F32):
        return nc.dram_tensor(name, list(shape), dt, kind="ExternalInput").ap()

    # inputs (per-core host layouts)
    xq8_d = din("xq8", (128, 4, 2, TQ), FP8)
    xk8_d = din("xk8", (128, 4, 2, TQ), FP8)
    xv8_d = din("xv8", (128, 4, 2, TQ), FP8)
    xqt_d = din("xqt", (128, 8, TQ))           # 1024*q^T feat-major f32
    wq8_d = din("wq8", (128, 4, 2, H), FP8)
    wk8_d = din("wk8", (128, 4, 2, H), FP8)
    wv8_d = din("wv8", (128, 4, 2, H), FP8)
    wo8_d = din("wo8", (64, 16, H), FP8)
    w18_d = din("w18", (128, 4, 2, 4 * H), FP8)
    w1l_d = din("w1l", (128, 4, 2, 4 * H), FP8)
    w28_d = din("w28", (128, 32, H), BF16)
    b1t_d = din("b1t", (128, 32))
    kb_d = din("kb", (128, JB))                # 0 / MASK_NEG per (t', j)
    s1m_d = din("s1m", (128, JB))              # mask*EXPSC/2 (offload path)
    s2m_d = din("s2m", (128, JB))              # mask (offload path)
    mrhs_d = din("mrhs", (128, 2, T), FP8)     # row0 pair0: 240*(1-qmask)
    s1c_d = din("s1c", (128, 8))               # 32*g1 per-feature cols
    s2c_d = din("s2c", (128, 8))               # 32*(be1+b2) cols
    # s = 32*(h + ff) feat-major; host does LN2 exactly
    out_d = nc.dram_tensor("out", [128, 8, TQ], F32,
                           kind="ExternalOutput").ap()

    def mmr(ap):
        return ap.bitcast(F32R)

    with tile.TileContext(nc) as tc, ExitStack() as top:
        # ---------------- persistent pools ----------------
        consts = top.enter_context(tc.tile_pool(name="consts", bufs=1))
        kb_sb = consts.tile([128, JB], F32, tag="kb", name="kb")
        s1m_sb = consts.tile([128, JB], F32, tag="s1m", name="s1m")
        s2m_sb = consts.tile([128, JB], F32, tag="s2m", name="s2m")
        b1t_sb = consts.tile([128, 32], F32, tag="b1t", name="b1t")
        s1c_sb = consts.tile([128, 8], F32, tag="s1c", name="s1c")
        s2c_sb = consts.tile([128, 8], F32, tag="s2c", name="s2c")
        mrhs_sb = consts.tile([128, 2, T], FP8, tag="mrhs", name="mrhs")
        mz_sb = consts.tile([128, 2, 96], FP8, tag="mz", name="mz")
        ones_f = consts.tile([128, 128], F32R, tag="ones", name="ones")
        eps1c = consts.tile([128, 1], F32, tag="e1", name="e1")

        # PSUM rings
        st_ps = tc.alloc_tile_pool(name="stps", bufs=2, space="PSUM")
        fps_ps = tc.alloc_tile_pool(name="fpsps", bufs=2, space="PSUM")
        f2_ps = tc.alloc_tile_pool(name="f2ps", bufs=2, space="PSUM")

        # attention-persistent SBUF
        att_pool = tc.alloc_tile_pool(name="att", bufs=1)
        v8 = [att_pool.tile([128, JB, 72], FP8, tag=f"v8{w}", name="v8")
              for w in range(NW)]
        Qp = att_pool.tile([32, 2, 8192], FP8, tag="Qp", name="Qp")
        Kp = att_pool.tile([32, 2, 8192], FP8, tag="Kp", name="Kp")
        w1_pool = tc.alloc_tile_pool(name="w1p", bufs=1)
        w18_sb = w1_pool.tile([128, 4, 2, 4 * H], FP8, tag="w18", name="w18")
        w1l_sb = w1_pool.tile([128, 4, 2, 4 * H], FP8, tag="w1l", name="w1l")
        wo_pool = tc.alloc_tile_pool(name="wop", bufs=1)
        wo8 = wo_pool.tile([64, 16, H], FP8, tag="wo8", name="wo8")

        # ---------------- input DMAs: critical path first ----------------
        qtkt_pool = tc.alloc_tile_pool(name="qtkt", bufs=1)
        # free layout (w, jh, t) so the repack DMA APs balance to 3 dims
        qT = qtkt_pool.tile([128, 4, 8, 128], FP8, tag="qT", name="qT")
        kT = qtkt_pool.tile([128, 4, 8, 128], FP8, tag="kT", name="kT")

        with tc.tile_pool(name="wproj", bufs=1) as wproj, \
             tc.tile_pool(name="xin", bufs=1) as xin:
            wq8 = wproj.tile([128, 4, 2, H], FP8, tag="wq", name="wq")
            xq8 = xin.tile([128, 4, 2, TQ], FP8, tag="xq", name="xq")
            wk8 = wproj.tile([128, 4, 2, H], FP8, tag="wk", name="wk")
            xk8 = xin.tile([128, 4, 2, TQ], FP8, tag="xk", name="xk")
            wv8 = wproj.tile([128, 4, 2, H], FP8, tag="wv", name="wv")
            xv8 = xin.tile([128, 4, 2, TQ], FP8, tag="xv", name="xv")
            # SP HWDGE queue: critical QKV inputs
            nc.sync.dma_start(wq8[:], wq8_d[:])
            nc.sync.dma_start(xq8[:], xq8_d[:])
            nc.sync.dma_start(wk8[:], wk8_d[:])
            nc.sync.dma_start(xk8[:], xk8_d[:])
            nc.sync.dma_start(wv8[:], wv8_d[:])
            nc.sync.dma_start(xv8[:], xv8_d[:])
            # ACT HWDGE queue: the flag rhs (needed by the first AV)
            nc.scalar.dma_start(mrhs_sb[:], mrhs_d[:])
            # SWDGE (Pool seq): consts off critical path
            nc.gpsimd.dma_start(kb_sb[:], kb_d[:])
            nc.gpsimd.dma_start(s1m_sb[:], s1m_d[:])
            nc.gpsimd.dma_start(s2m_sb[:], s2m_d[:])
            nc.gpsimd.dma_start(b1t_sb[:], b1t_d[:])
            nc.gpsimd.dma_start(s1c_sb[:], s1c_d[:])
            nc.gpsimd.dma_start(s2c_sb[:], s2c_d[:])
            nc.gpsimd.memset(mz_sb[:], 0.0)
            nc.gpsimd.memset(mz_sb[0:1, 0:1, 64:65], 240.0)
            nc.vector.memset(ones_f[:].bitcast(F32), 1.0)
            nc.vector.memset(eps1c[:], EPS1)
            for w in range(NW):
                nc.gpsimd.memset(v8[w][:, :, 64:65], 1.0)

            # ============ P1: projections (fp8 DoubleRow) ============
            for (wt, xt, dst) in ((wq8, xq8, qT), (wk8, xk8, kT)):
                for ht in range(8):
                    ps = f2_ps.tile([128, TQ], F32, tag="f2", name="pp")
                    for kt in range(4):
                        nc.tensor.matmul(
                            ps[:], wt[:, kt, :, 128 * ht:128 * (ht + 1)],
                            xt[:, kt], start=(kt == 0), stop=(kt == 3),
                            perf_mode=DR)
                    nc.vector.tensor_copy(
                        dst[:, :, ht, :],
                        ps[:].rearrange("p (w t) -> p w t", t=128))
            for tt in range(NW):
                for nk in range(2):
                    ps = f2_ps.tile([128, TQ], F32, tag="f2", name="pp")
                    for kt in range(4):
                        nc.tensor.matmul(
                            ps[:], xv8[:, kt, :, 128 * tt:128 * (tt + 1)],
                            wv8[:, kt, :, 512 * nk:512 * (nk + 1)],
                            start=(kt == 0), stop=(kt == 3), perf_mode=DR)
                    nc.vector.tensor_copy(
                        v8[tt][:, 8 * nk:8 * (nk + 1), 0:64],
                        ps[:].rearrange("p (a b) -> p a b", b=64))

        # ============ P1.5: repack Q/K -> [32, 2, (w, j, t')] ============
        # 8 consolidated DMAs (one per src x i x hh), covering all windows.
        for (src, dst) in ((qT, Qp), (kT, Kp)):
            for i in range(2):
                for hh in range(2):
                    s_ap = src[32 * i + 64 * hh:32 * i + 64 * hh + 32]
                    d_ap = dst[:, i] \
                        .rearrange("p (w jh two t) -> p w jh two t",
                                   w=4, jh=8, two=2, t=128)[:, :, :, hh, :]
                    nc.sync.dma_start(d_ap, s_ap)
        qtkt_pool.release()

        # pools created after the P1 input pools release their SBUF
        probs_pool = tc.alloc_tile_pool(name="probs", bufs=8)
        hp_pool = tc.alloc_tile_pool(name="hp", bufs=2)
        toff_pool = tc.alloc_tile_pool(name="toff", bufs=2)
        rbc_pool = tc.alloc_tile_pool(name="rbc", bufs=2)
        xqt_pool = tc.alloc_tile_pool(name="xqtp", bufs=1)
        xp_pool = tc.alloc_tile_pool(name="xp", bufs=1)
        hs_pool = tc.alloc_tile_pool(name="hs", bufs=1)
        zg_pool = tc.alloc_tile_pool(name="zg", bufs=1)
        zc_pool = tc.alloc_tile_pool(name="zc", bufs=2)
        w2s_pool = tc.alloc_tile_pool(name="w2s", bufs=5)
        sdr_pool = tc.alloc_tile_pool(name="sdr", bufs=2)
        tt1_pool = tc.alloc_tile_pool(name="tt1", bufs=2)
        lnt_pool = tc.alloc_tile_pool(name="lnt", bufs=1)
        xsq_pool = tc.alloc_tile_pool(name="xsq", bufs=1)

        # ---------------- stage emitters ----------------
        def attn_window(w):
            P_ = w // 2
            wh = w % 2
            if wh == 0:
                xqt_t = xqt_pool.tile([128, 8, 256], F32, tag="xqt",
                                      name="xqt")
                nc.scalar.dma_start(xqt_t[:],
                                    xqt_d[:, :, 256 * P_:256 * (P_ + 1)])
                x_t = xp_pool.tile([128, 8, 256], F32R, tag="x", name="x")
                _state["xqt"] = xqt_t
                _state["x"] = x_t
            else:
                xqt_t = _state["xqt"]
                x_t = _state["x"]
            hp = hp_pool.tile([64, T], FP8, tag="hp", name="hp")
            for half in range(2):
                pairs = []
                for jk in range(JB):
                    if jk % 2 == 0:
                        pairs.append(probs_pool.tile([128, 2, 1024], FP8,
                                                     tag="pr", name="pr"))
                    pr = pairs[jk // 2][:, jk % 2, :]
                    strip = st_ps.tile([128, 1024], F32, tag="st",
                                       name="st")
                    for qc in range(2):
                        qoff = 2048 * w + 1024 * half + 512 * qc
                        nc.tensor.matmul(
                            strip[:, 512 * qc:512 * (qc + 1)],
                            Kp[:, :, 2048 * w + 128 * jk:
                               2048 * w + 128 * (jk + 1)],
                            Qp[:, :, qoff:qoff + 512],
                            start=True, stop=True, perf_mode=DR,
                            skip_group_check=True)
                    if _offload(w, jk, half):
                        t_sb = toff_pool.tile([128, 1024], BF16, tag="t",
                                              name="t")
                        nc.vector.tensor_scalar(
                            out=t_sb[:], in0=strip[:],
                            scalar1=s1m_sb[:, jk:jk + 1],
                            scalar2=s2m_sb[:, jk:jk + 1],
                            op0=OP.mult, op1=OP.add)
                        nc.gpsimd.tensor_mul(pr, t_sb[:], t_sb[:])
                    else:
                        nc.scalar.activation(pr, strip[:], AF.Exp,
                                             bias=kb_sb[:, jk:jk + 1],
                                             scale=EXPSC)
                pavs = [fps_ps.tile([128, 512], F32, tag="fps", name="pav")
                        for _ in range(2)]
                for m in range(8):
                    for qc in range(2):
                        sl = slice(512 * qc, 512 * (qc + 1))
                        nc.tensor.matmul(
                            pavs[qc][0:65, :],
                            v8[w][:, 2 * m:2 * m + 2, 0:65],
                            pairs[m][:, :, sl],
                            start=(m == 0), stop=False, perf_mode=DR,
                            skip_group_check=True)
                for qc in range(2):
                    moff = 1024 * half + 512 * qc
                    nc.tensor.matmul(
                        pavs[qc][0:65, :], mz_sb[:, :, 0:65],
                        mrhs_sb[:, :, moff:moff + 512],
                        start=False, stop=True, perf_mode=DR,
                        skip_group_check=True)
                    # division: recip row -> Pool broadcast -> DVE mul
                    rbc = rbc_pool.tile([64, 512], BF16, tag="rbc",
                                        name="rbc")
                    with nc.allow_low_precision(reason="softmax recip bf16"):
                        nc.vector.reciprocal(rbc[0:1, :], pavs[qc][64:65, :])
                    nc.gpsimd.partition_broadcast(rbc[:], rbc[0:1, :])
                    nc.vector.tensor_mul(
                        hp[:, 1024 * half + 512 * qc:
                           1024 * half + 512 * (qc + 1)],
                        pavs[qc][0:64, :], rbc[:])
            # ---- Wo(w) + residual ----
            wops = st_ps.tile([128, 1024], F32, tag="st", name="wo")
            for fo in range(8):
                for m in range(8):
                    nc.tensor.matmul(
                        wops[:, 128 * fo:128 * (fo + 1)],
                        wo8[:, 2 * m:2 * m + 2, 128 * fo:128 * (fo + 1)],
                        hp[:].rearrange("p (j t) -> p j t", t=128)
                        [:, 2 * m:2 * m + 2, :],
                        start=(m == 0), stop=(m == 7), perf_mode=DR,
                        skip_group_check=True)
            nc.vector.tensor_add(
                x_t[:, :, 128 * wh:128 * (wh + 1)],
                wops[:].rearrange("p (a b) -> p a b", b=128),
                xqt_t[:, :, 128 * wh:128 * (wh + 1)])

        def ln1_ffn1_gelu(P):
            x_t = _state["x"]
            h_t = hs_pool.tile([128, 8, 256], BF16, tag="h", name="h")
            h8_t = hs_pool.tile([128, 4, 2, 256], FP8, tag="h8", name="h8")
            # ---- LN1 stats (fp32r ones-matmuls) ----
            smu = fps_ps.tile([128, 256], F32, tag="fps", name="smu")
            svar = fps_ps.tile([128, 256], F32, tag="fps", name="svar")
            for fo in range(8):
                xsq = xsq_pool.tile([128, 256], F32R, tag="xsq", name="xsq")
                nc.gpsimd.tensor_mul(xsq[:], x_t[:, fo, :], x_t[:, fo, :])
                nc.tensor.matmul(smu[:], ones_f[:], x_t[:, fo, :],
                                 start=(fo == 0), stop=(fo == 7),
                                 skip_group_check=True)
                nc.tensor.matmul(svar[:], ones_f[:], xsq[:],
                                 start=(fo == 0), stop=(fo == 7),
                                 skip_group_check=True)
            mu_s = lnt_pool.tile([128, 256], F32, tag="mu", name="mu")
            nc.vector.tensor_scalar_mul(mu_s[:], smu[:], 1.0 / H)
            var = lnt_pool.tile([128, 256], F32, tag="var", name="var")
            nc.vector.tensor_scalar_mul(var[:], svar[:], 1.0 / H)
            mu2 = lnt_pool.tile([128, 256], F32, tag="mu2", name="mu2")
            nc.vector.tensor_mul(mu2[:], mu_s[:], mu_s[:])
            nc.vector.tensor_sub(var[:], var[:], mu2[:])
            rstd = lnt_pool.tile([128, 256], F32, tag="rst", name="rst")
            nc.scalar.activation(rstd[:], var[:], AF.Sqrt, bias=eps1c[:])
            nc.vector.reciprocal(rstd[:], rstd[:])
            for fo in range(8):
                t1 = lnt_pool.tile([128, 256], F32, tag="t1", name="t1")
                nc.vector.tensor_sub(t1[:], x_t[:, fo, :], mu_s[:])
                nc.vector.tensor_mul(h_t[:, fo, :], t1[:], rstd[:])
                nc.gpsimd.tensor_copy(h8_t[:, fo // 2, fo % 2, :],
                                      h_t[:, fo, :])
            _state["h"] = h_t
            # ---- FFN1: resident W1 (fp8 + fp8 residual), z = fps/32+b1 ----
            if P == 0:
                G_b = zg_pool.tile([128, 32, 256], BF16, tag="Gb", name="Gb")
                _state["G"] = G_b
            else:
                G_b = _state["G"]
            zch = None
            for ot in range(32):
                if ot % 4 == 0:
                    zch = zc_pool.tile([128, 4, 256], BF16, tag="zc",
                                       name="zc")
                fps = fps_ps.tile([128, 256], F32, tag="fps", name="f1")
                for kt in range(4):
                    nc.tensor.matmul(
                        fps[:], w18_sb[:, kt, :, 128 * ot:128 * (ot + 1)],
                        h8_t[:, kt, :, :], start=(kt == 0),
                        stop=False, perf_mode=DR)
                for kt in range(4):
                    nc.tensor.matmul(
                        fps[:], w1l_sb[:, kt, :, 128 * ot:128 * (ot + 1)],
                        h8_t[:, kt, :, :], start=False,
                        stop=(kt == 3), perf_mode=DR)
                with nc.allow_low_precision(reason="pre-gelu bf16"):
                    nc.vector.tensor_scalar(
                        out=zch[:, ot % 4, :], in0=fps[:],
                        scalar1=1.0 / SW,
                        scalar2=b1t_sb[:, ot:ot + 1],
                        op0=OP.mult, op1=OP.add)
                if ot % 4 == 3:
                    gi = ot // 4
                    nc.scalar.activation(
                        G_b[:, 4 * gi:4 * (gi + 1), :],
                        zch[:], AF.Gelu, scale=1.0)

        def ffn2_pass(P, fp):
            G_b = _state["G"]
            h_t = _state["h"]
            f2 = [f2_ps.tile([128, 256], F32, tag="f2", name="f2")
                  for _ in range(2)]
            for c4 in range(8):
                w2c = w2s_pool.tile([128, 4, 256], BF16, tag="w2c",
                                    name="w2c")
                nc.sync.dma_start(
                    w2c[:], w28_d[:, 4 * c4:4 * (c4 + 1),
                                  256 * fp:256 * (fp + 1)])
                for ktl in range(4):
                    for fh in range(2):
                        nc.tensor.matmul(
                            f2[fh][:],
                            w2c[:, ktl, 128 * fh:128 * (fh + 1)],
                            G_b[:, 4 * c4 + ktl, :],
                            start=(c4 == 0 and ktl == 0),
                            stop=(c4 == 7 and ktl == 3),
                            skip_group_check=True)
            # s = f2 + 32*h*g1 + 32*(be1+b2)  (f32, exact for host LN2)
            tt1 = tt1_pool.tile([128, 512], BF16, tag="tt", name="tt")
            s_t = sdr_pool.tile([128, 512], F32, tag="s", name="s")
            for fh in range(2):
                fo = 2 * fp + fh
                nc.vector.tensor_scalar(
                    out=tt1[:, 256 * fh:256 * (fh + 1)],
                    in0=h_t[:, fo, :],
                    scalar1=s1c_sb[:, fo:fo + 1],
                    scalar2=s2c_sb[:, fo:fo + 1],
                    op0=OP.mult, op1=OP.add)
                nc.vector.tensor_add(s_t[:, 256 * fh:256 * (fh + 1)],
                                     f2[fh][:],
                                     tt1[:, 256 * fh:256 * (fh + 1)])
            nc.sync.dma_start(
                out_d[:, 2 * fp:2 * (fp + 1), :]
                .rearrange("p a (c t) -> p c a t", c=2)[:, P],
                s_t[:].rearrange("p (a t) -> p a t", a=2))

        # ---------------- pipelined emission ----------------
        nc.sync.dma_start(wo8[:], wo8_d[:])
        # W1 prefetch (8MB) on the SP queue right after the repack: its
        # transfers fill the DMA-idle window during the w0/w1 exps and are
        # resident well before FFN1(P0)
        for c in range(4):
            nc.sync.dma_start(w18_sb[:, c], w18_d[:, c])
        for c in range(4):
            nc.sync.dma_start(w1l_sb[:, c], w1l_d[:, c])
        attn_window(0)
        attn_window(1)
        ln1_ffn1_gelu(0)
        attn_window(2)
        ffn2_pass(0, 0)
        ffn2_pass(0, 1)
        attn_window(3)
        ffn2_pass(0, 2)
        ffn2_pass(0, 3)
        ln1_ffn1_gelu(1)
        for fp in range(4):
            ffn2_pass(1, fp)

        for p in (xsq_pool, lnt_pool, tt1_pool, sdr_pool, w2s_pool,
                  zc_pool, zg_pool, hs_pool, xp_pool, xqt_pool, rbc_pool,
                  toff_pool, hp_pool, probs_pool, wo_pool, w1_pool,
                  att_pool, f2_ps, fps_ps, st_ps):
            p.release()

    nc.compile()
    return nc


_state = {}
_nc_cache = {}


def get_nc(key="full"):
    if key not in _nc_cache:
        _state.clear()
        _nc_cache[key] = build_program()
    return _nc_cache[key]


def host_prep(q, k, v, pad_mask, Wq, Wk, Wv, Wo, W1, b1, W2, b2,
              g1, be1, g2, be2):
    import ml_dtypes
    f = np.float32
    NPFP8 = ml_dtypes.float8_e4m3
    asf = lambda a: np.asarray(a, dtype=f)
    q, k, v = asf(q), asf(k), asf(v)
    pad = np.asarray(pad_mask)
    g1, be1, b1, b2 = asf(g1), asf(be1), asf(b1), asf(b2)

    def to8(a):
        return np.ascontiguousarray(a).astype(NPFP8)

    def wlay(Wmat, scale):  # [out, in] -> [128, in/256, 2, out] fp8
        wT = np.ascontiguousarray(asf(Wmat).T) * scale
        n_in, n_out = wT.shape
        return to8(wT.reshape(n_in // 256, 2, 128, n_out)
                   .transpose(2, 0, 1, 3))

    wq8 = wlay(Wq, SW)
    wk8 = wlay(Wk, SW)
    wv8 = wlay(Wv, SW)
    woT = np.ascontiguousarray(asf(Wo).T) * SW          # [in, out]
    wo8 = to8(woT.reshape(16, 64, H).transpose(1, 0, 2))
    W1p = asf(W1) * g1[None, :]                         # fold g1
    w18 = wlay(W1p, SW)
    w1res = (asf(W1p).T * SW) - \
        w18.transpose(1, 2, 0, 3).reshape(H, 4 * H).astype(f)
    w1l = to8(w1res.reshape(4, 2, 128, 4 * H).transpose(2, 0, 1, 3))
    import ml_dtypes as _mld
    w2T = np.ascontiguousarray(asf(W2).T) * SW
    w28 = np.ascontiguousarray(
        w2T.reshape(32, 128, H).transpose(1, 0, 2)).astype(_mld.bfloat16)
    b1p = b1 + asf(W1) @ be1                            # fold be1
    b1t = np.ascontiguousarray(b1p.reshape(32, 128).T)
    s1c = np.ascontiguousarray((SW * g1).reshape(8, 128).T)
    s2c = np.ascontiguousarray((SW * (be1 + b2)).reshape(8, 128).T)

    in_maps = []
    for c in range(NCORES):
        b_, s_ = c // GROUP, c % GROUP
        sl = slice(s_ * TQ, (s_ + 1) * TQ)
        pm = pad[b_].reshape(128, JB).astype(f)         # [t', j]
        kb = np.where(pm > 0, f(0.0), f(MASK_NEG)).astype(f)
        s1m = np.ascontiguousarray(pm * (EXPSC / 2))
        s2m = np.ascontiguousarray(pm)
        qm = np.ascontiguousarray(pm.T).reshape(-1)     # [128j + t']
        mrhs = np.zeros((128, 2, T), f)
        mrhs[0, 0, :] = 240.0 * (1.0 - qm)
        def xlay(x):
            xT = np.ascontiguousarray(x[b_, sl].T)      # [H, TQ]
            return to8(xT.reshape(4, 2, 128, TQ).transpose(2, 0, 1, 3))
        xqt = np.ascontiguousarray(
            (q[b_, sl].T * SQ).reshape(8, 128, TQ).transpose(1, 0, 2))
        in_maps.append(dict(
            xq8=xlay(q), xk8=xlay(k), xv8=xlay(v),
            xqt=np.ascontiguousarray(xqt, dtype=f),
            wq8=wq8, wk8=wk8, wv8=wv8, wo8=wo8, w18=w18, w1l=w1l,
            w28=w28,
            b1t=b1t, kb=np.ascontiguousarray(kb),
            s1m=s1m, s2m=s2m, mrhs=to8(mrhs), s1c=s1c, s2c=s2c,
        ))
    return in_maps


def kernel(q, k, v, pad_mask, Wq, Wk, Wv, Wo, W1, b1, W2, b2,
           g1, be1, g2, be2):
    from concourse.bass_utils import run_bass_kernel_spmd

    nc = get_nc()
    in_maps = host_prep(q, k, v, pad_mask, Wq, Wk, Wv, Wo, W1, b1, W2, b2,
                        g1, be1, g2, be2)
    res = run_bass_kernel_spmd(nc, in_maps, core_ids=list(range(NCORES)))
    g2f = np.asarray(g2, np.float32)
    be2f = np.asarray(be2, np.float32)
    out = np.empty((B, T, H), np.float32)
    eps2 = EPS * SW * SW
    for c in range(NCORES):
        b_, s_ = c // GROUP, c % GROUP
        s_fm = res.results[c]["out"]                    # [128, 8, TQ]
        s_tok = np.transpose(s_fm, (2, 1, 0)).reshape(TQ, H)
        mu = s_tok.mean(axis=1, keepdims=True)
        var = s_tok.var(axis=1, keepdims=True)
        o = (s_tok - mu) / np.sqrt(var + eps2)
        out[b_, s_ * TQ:(s_ + 1) * TQ, :] = \
            o * g2f[None, :] + be2f[None, :]
    return out


# revision 41
# speedup vs baseline: 1.0356x; 1.0356x over previous
"""Trainium2 Bass kernel for nn_MultiHeadAttention_51986284150861 (fp8 rev).

Full transformer block (MHA + LN1 + FFN; LN2 applied exactly on host) with
the reference's faithful torch-.view head split: head a attends over 2048
positions (j, t') where position (j, t') draws features 64j..64j+63 of
token 128a + t'.  Per core: batch b = c//4, 512 tokens = 4 head windows.
Zero collectives.

Pipelined emission so the ready-heap tile scheduler overlaps the ACT-bound
softmax of pair P+1 with the PE-bound FFN of pair P:
  attn(w0) attn(w1) | LN1(0) FFN1(0) gelu(0) | attn(w2) ~ FFN2(0) ~ attn(w3)
  | LN1(1) FFN1(1) gelu(1) FFN2(1)
PSUM rings: strips/wops [128,1024]x2 (4 banks), fps/stats [128,256]x2
(2 banks), f2/pav [128,512]x2 (2 banks).  FFN2 runs as 4 fo-pair passes
(W2 streamed once per pair).  FFN1 pre-gelu drains to SBUF (DVE/ACT
split), gelu batched as [128,1024] tiles to bound ACT table switches.
Softmax division: DVE recip -> Pool partition_broadcast -> DVE mul.
"""

import os
import sys

sys.path.insert(0, "/opt/trn_rl_repo")

from contextlib import ExitStack

import numpy as np

import concourse.bass as bass
import concourse.tile as tile
from concourse import bacc, mybir

F32 = mybir.dt.float32
F32R = mybir.dt.float32r
BF16 = mybir.dt.bfloat16
FP8 = mybir.dt.float8e4
DR = mybir.MatmulPerfMode.DoubleRow
AF = mybir.ActivationFunctionType
OP = mybir.AluOpType

B, T, H, NH, D = 2, 2048, 1024, 16, 64
EPS = 1e-5
NCORES = 8
GROUP = NCORES // B
TQ = T // GROUP              # 512 tokens/core
NW = 4                       # head windows per core
JB = 16                      # feature blocks
SW = 32.0                    # fp8 weight scale
SQ = 1024.0                  # q residual prescale (= SW*SW)
EXPSC = 1.0 / (8.0 * SW * SW)
MASK_NEG = -80.0
EPS1 = EPS * SQ * SQ         # LN1 eps (x is 1024-scaled)

# (w, jk, half) triples whose softmax goes DVE-affine + Pool-square instead
# of ACT exp.
N_OFF = int(os.environ.get("MHA_N_OFF", "0"))
OFF_JKS = [1, 3, 5, 7, 9, 11, 2, 6, 10, 14]
OFF_WINS = int(os.environ.get("MHA_OFF_WINS", "0"))
OFFLOAD = {(w, jk, half) for w in range(OFF_WINS) for half in range(2)
           for jk in OFF_JKS[:N_OFF]}
# FFN1 pre-gelu drains: which ot indices drain via ACT Copy (rest on DVE)
ACT_Z = int(os.environ.get("MHA_ACT_Z", "0"))  # of every 4


def _offload(w, jk, half):
    return (w, jk, half) in OFFLOAD


def build_program():
    nc = bacc.Bacc("TRN2", target_bir_lowering=False, debug=False)

    def din(name, shape, dt=F32):
        return nc.dram_tensor(name, list(shape), dt, kind="ExternalInput").ap()

    # inputs (per-core host layouts)
    xq8_d = din("xq8", (128, 4, 2, TQ), FP8)
    xk8_d = din("xk8", (128, 4, 2, TQ), FP8)
    xv8_d = din("xv8", (128, 4, 2, TQ), FP8)
    xqt_d = din("xqt", (128, 8, TQ))           # 1024*q^T feat-major f32
    wq8_d = din("wq8", (128, 4, 2, H), FP8)
    wk8_d = din("wk8", (128, 4, 2, H), FP8)
    wv8_d = din("wv8", (128, 4, 2, H), FP8)
    wo8_d = din("wo8", (64, 16, H), FP8)
    w18_d = din("w18", (128, 4, 2, 4 * H), FP8)
    w1l_d = din("w1l", (128, 4, 2, 4 * H), FP8)
    w28_d = din("w28", (128, 32, H), BF16)
    b1t_d = din("b1t", (128, 32))
    kb_d = din("kb", (128, JB))                # 0 / MASK_NEG per (t', j)
    s1m_d = din("s1m", (128, JB))              # mask*EXPSC/2 (offload path)
    s2m_d = din("s2m", (128, JB))              # mask (offload path)
    mrhs_d = din("mrhs", (128, 2, T), FP8)     # row0 pair0: 240*(1-qmask)
    s1c_d = din("s1c", (128, 8))               # 32*g1 per-feature cols
    s2c_d = din("s2c", (128, 8))               # 32*(be1+b2) cols
    # s = 32*(h + ff) feat-major; host does LN2 exactly
    out_d = nc.dram_tensor("out", [128, 8, TQ], F32,
                           kind="ExternalOutput").ap()

    def mmr(ap):
        return ap.bitcast(F32R)

    with tile.TileContext(nc) as tc, ExitStack() as top:
        # ---------------- persistent pools ----------------
        consts = top.enter_context(tc.tile_pool(name="consts", bufs=1))
        kb_sb = consts.tile([128, JB], F32, tag="kb", name="kb")
        s1m_sb = consts.tile([128, JB], F32, tag="s1m", name="s1m")
        s2m_sb = consts.tile([128, JB], F32, tag="s2m", name="s2m")
        b1t_sb = consts.tile([128, 32], F32, tag="b1t", name="b1t")
        s1c_sb = consts.tile([128, 8], F32, tag="s1c", name="s1c")
        s2c_sb = consts.tile([128, 8], F32, tag="s2c", name="s2c")
        mrhs_sb = consts.tile([128, 2, T], FP8, tag="mrhs", name="mrhs")
        mz_sb = consts.tile([128, 2, 96], FP8, tag="mz", name="mz")
        ones_f = consts.tile([128, 128], F32R, tag="ones", name="ones")
        eps1c = consts.tile([128, 1], F32, tag="e1", name="e1")

        # PSUM rings
        st_ps = tc.alloc_tile_pool(name="stps", bufs=2, space="PSUM")
        fps_ps = tc.alloc_tile_pool(name="fpsps", bufs=2, space="PSUM")
        f2_ps = tc.alloc_tile_pool(name="f2ps", bufs=2, space="PSUM")

        # attention-persistent SBUF
        att_pool = tc.alloc_tile_pool(name="att", bufs=1)
        v8 = [att_pool.tile([128, JB, 96], FP8, tag=f"v8{w}", name="v8")
              for w in range(NW)]
        Qp = att_pool.tile([32, 2, 8192], FP8, tag="Qp", name="Qp")
        Kp = att_pool.tile([32, 2, 8192], FP8, tag="Kp", name="Kp")
        w1_pool = tc.alloc_tile_pool(name="w1p", bufs=1)
        w18_sb = w1_pool.tile([128, 4, 2, 4 * H], FP8, tag="w18", name="w18")
        w1l_sb = w1_pool.tile([128, 4, 2, 4 * H], FP8, tag="w1l", name="w1l")
        wo_pool = tc.alloc_tile_pool(name="wop", bufs=1)
        wo8 = wo_pool.tile([64, 16, H], FP8, tag="wo8", name="wo8")

        # ---------------- input DMAs: critical path first ----------------
        qtkt_pool = tc.alloc_tile_pool(name="qtkt", bufs=1)
        # free layout (w, jh, t) so the repack DMA APs balance to 3 dims
        qT = qtkt_pool.tile([128, 4, 8, 128], FP8, tag="qT", name="qT")
        kT = qtkt_pool.tile([128, 4, 8, 128], FP8, tag="kT", name="kT")

        with tc.tile_pool(name="wproj", bufs=1) as wproj, \
             tc.tile_pool(name="xin", bufs=1) as xin:
            wq8 = wproj.tile([128, 4, 2, H], FP8, tag="wq", name="wq")
            xq8 = xin.tile([128, 4, 2, TQ], FP8, tag="xq", name="xq")
            wk8 = wproj.tile([128, 4, 2, H], FP8, tag="wk", name="wk")
            xk8 = xin.tile([128, 4, 2, TQ], FP8, tag="xk", name="xk")
            wv8 = wproj.tile([128, 4, 2, H], FP8, tag="wv", name="wv")
            xv8 = xin.tile([128, 4, 2, TQ], FP8, tag="xv", name="xv")
            # SP HWDGE queue: critical QKV inputs
            nc.sync.dma_start(wq8[:], wq8_d[:])
            nc.sync.dma_start(xq8[:], xq8_d[:])
            nc.sync.dma_start(wk8[:], wk8_d[:])
            nc.sync.dma_start(xk8[:], xk8_d[:])
            nc.sync.dma_start(wv8[:], wv8_d[:])
            nc.sync.dma_start(xv8[:], xv8_d[:])
            # ACT HWDGE queue: the flag rhs (needed by the first AV)
            nc.scalar.dma_start(mrhs_sb[:], mrhs_d[:])
            # SWDGE (Pool seq): consts off critical path
            nc.gpsimd.dma_start(kb_sb[:], kb_d[:])
            nc.gpsimd.dma_start(s1m_sb[:], s1m_d[:])
            nc.gpsimd.dma_start(s2m_sb[:], s2m_d[:])
            nc.gpsimd.dma_start(b1t_sb[:], b1t_d[:])
            nc.gpsimd.dma_start(s1c_sb[:], s1c_d[:])
            nc.gpsimd.dma_start(s2c_sb[:], s2c_d[:])
            nc.gpsimd.memset(mz_sb[:], 0.0)
            nc.gpsimd.memset(mz_sb[0:1, 0:1, 64:65], 240.0)
            nc.vector.memset(ones_f[:].bitcast(F32), 1.0)
            nc.vector.memset(eps1c[:], EPS1)
            for w in range(NW):
                nc.gpsimd.memset(v8[w][:, :, 64:65], 1.0)

            # ============ P1: projections (fp8 DoubleRow) ============
            for (wt, xt, dst) in ((wq8, xq8, qT), (wk8, xk8, kT)):
                for ht in range(8):
                    ps = f2_ps.tile([128, TQ], F32, tag="f2", name="pp")
                    for kt in range(4):
                        nc.tensor.matmul(
                            ps[:], wt[:, kt, :, 128 * ht:128 * (ht + 1)],
                            xt[:, kt], start=(kt == 0), stop=(kt == 3),
                            perf_mode=DR)
                    nc.vector.tensor_copy(
                        dst[:, :, ht, :],
                        ps[:].rearrange("p (w t) -> p w t", t=128))
            for tt in range(NW):
                for nk in range(2):
                    ps = f2_ps.tile([128, TQ], F32, tag="f2", name="pp")
                    for kt in range(4):
                        nc.tensor.matmul(
                            ps[:], xv8[:, kt, :, 128 * tt:128 * (tt + 1)],
                            wv8[:, kt, :, 512 * nk:512 * (nk + 1)],
                            start=(kt == 0), stop=(kt == 3), perf_mode=DR)
                    nc.vector.tensor_copy(
                        v8[tt][:, 8 * nk:8 * (nk + 1), 0:64],
                        ps[:].rearrange("p (a b) -> p a b", b=64))

        # ============ P1.5: repack Q/K -> [32, 2, (w, j, t')] ============
        # 8 consolidated DMAs (one per src x i x hh), covering all windows.
        for (src, dst) in ((qT, Qp), (kT, Kp)):
            for i in range(2):
                for hh in range(2):
                    s_ap = src[32 * i + 64 * hh:32 * i + 64 * hh + 32]
                    d_ap = dst[:, i] \
                        .rearrange("p (w jh two t) -> p w jh two t",
                                   w=4, jh=8, two=2, t=128)[:, :, :, hh, :]
                    nc.sync.dma_start(d_ap, s_ap)
        qtkt_pool.release()

        # pools created after the P1 input pools release their SBUF
        probs_pool = tc.alloc_tile_pool(name="probs", bufs=8)
        hp_pool = tc.alloc_tile_pool(name="hp", bufs=2)
        toff_pool = tc.alloc_tile_pool(name="toff", bufs=2)
        rbc_pool = tc.alloc_tile_pool(name="rbc", bufs=2)
        xqt_pool = tc.alloc_tile_pool(name="xqtp", bufs=1)
        xp_pool = tc.alloc_tile_pool(name="xp", bufs=1)
        hs_pool = tc.alloc_tile_pool(name="hs", bufs=1)
        zg_pool = tc.alloc_tile_pool(name="zg", bufs=1)
        zc_pool = tc.alloc_tile_pool(name="zc", bufs=2)
        w2s_pool = tc.alloc_tile_pool(name="w2s", bufs=4)
        sdr_pool = tc.alloc_tile_pool(name="sdr", bufs=2)
        tt1_pool = tc.alloc_tile_pool(name="tt1", bufs=2)
        lnt_pool = tc.alloc_tile_pool(name="lnt", bufs=1)
        xsq_pool = tc.alloc_tile_pool(name="xsq", bufs=1)

        # ---------------- stage emitters ----------------
        def attn_window(w):
            P_ = w // 2
            wh = w % 2
            if wh == 0:
                xqt_t = xqt_pool.tile([128, 8, 256], F32, tag="xqt",
                                      name="xqt")
                nc.scalar.dma_start(xqt_t[:],
                                    xqt_d[:, :, 256 * P_:256 * (P_ + 1)])
                x_t = xp_pool.tile([128, 8, 256], F32R, tag="x", name="x")
                _state["xqt"] = xqt_t
                _state["x"] = x_t
            else:
                xqt_t = _state["xqt"]
                x_t = _state["x"]
            hp = hp_pool.tile([64, T], FP8, tag="hp", name="hp")
            for half in range(2):
                pairs = []
                for jk in range(JB):
                    if jk % 2 == 0:
                        pairs.append(probs_pool.tile([128, 2, 1024], FP8,
                                                     tag="pr", name="pr"))
                    pr = pairs[jk // 2][:, jk % 2, :]
                    strip = st_ps.tile([128, 1024], F32, tag="st",
                                       name="st")
                    for qc in range(2):
                        qoff = 2048 * w + 1024 * half + 512 * qc
                        nc.tensor.matmul(
                            strip[:, 512 * qc:512 * (qc + 1)],
                            Kp[:, :, 2048 * w + 128 * jk:
                               2048 * w + 128 * (jk + 1)],
                            Qp[:, :, qoff:qoff + 512],
                            start=True, stop=True, perf_mode=DR,
                            skip_group_check=True)
                    if _offload(w, jk, half):
                        t_sb = toff_pool.tile([128, 1024], BF16, tag="t",
                                              name="t")
                        nc.vector.tensor_scalar(
                            out=t_sb[:], in0=strip[:],
                            scalar1=s1m_sb[:, jk:jk + 1],
                            scalar2=s2m_sb[:, jk:jk + 1],
                            op0=OP.mult, op1=OP.add)
                        nc.gpsimd.tensor_mul(pr, t_sb[:], t_sb[:])
                    else:
                        nc.scalar.activation(pr, strip[:], AF.Exp,
                                             bias=kb_sb[:, jk:jk + 1],
                                             scale=EXPSC)
                pavs = [fps_ps.tile([128, 512], F32, tag="fps", name="pav")
                        for _ in range(2)]
                for m in range(8):
                    for qc in range(2):
                        sl = slice(512 * qc, 512 * (qc + 1))
                        nc.tensor.matmul(
                            pavs[qc][0:65, :],
                            v8[w][:, 2 * m:2 * m + 2, 0:65],
                            pairs[m][:, :, sl],
                            start=(m == 0), stop=False, perf_mode=DR,
                            skip_group_check=True)
                for qc in range(2):
                    moff = 1024 * half + 512 * qc
                    nc.tensor.matmul(
                        pavs[qc][0:65, :], mz_sb[:, :, 0:65],
                        mrhs_sb[:, :, moff:moff + 512],
                        start=False, stop=True, perf_mode=DR,
                        skip_group_check=True)
                    # division: recip row -> Pool broadcast -> DVE mul
                    rbc = rbc_pool.tile([64, 512], BF16, tag="rbc",
                                        name="rbc")
                    with nc.allow_low_precision(reason="softmax recip bf16"):
                        nc.vector.reciprocal(rbc[0:1, :], pavs[qc][64:65, :])
                    nc.gpsimd.partition_broadcast(rbc[:], rbc[0:1, :])
                    nc.vector.tensor_mul(
                        hp[:, 1024 * half + 512 * qc:
                           1024 * half + 512 * (qc + 1)],
                        pavs[qc][0:64, :], rbc[:])
            # ---- Wo(w) + residual ----
            wops = st_ps.tile([128, 1024], F32, tag="st", name="wo")
            for fo in range(8):
                for m in range(8):
                    nc.tensor.matmul(
                        wops[:, 128 * fo:128 * (fo + 1)],
                        wo8[:, 2 * m:2 * m + 2, 128 * fo:128 * (fo + 1)],
                        hp[:].rearrange("p (j t) -> p j t", t=128)
                        [:, 2 * m:2 * m + 2, :],
                        start=(m == 0), stop=(m == 7), perf_mode=DR,
                        skip_group_check=True)
            nc.vector.tensor_add(
                x_t[:, :, 128 * wh:128 * (wh + 1)],
                wops[:].rearrange("p (a b) -> p a b", b=128),
                xqt_t[:, :, 128 * wh:128 * (wh + 1)])

        def ln1_ffn1_gelu(P):
            x_t = _state["x"]
            h_t = hs_pool.tile([128, 8, 256], BF16, tag="h", name="h")
            h8_t = hs_pool.tile([128, 4, 2, 256], FP8, tag="h8", name="h8")
            # ---- LN1 stats (fp32r ones-matmuls) ----
            smu = fps_ps.tile([128, 256], F32, tag="fps", name="smu")
            svar = fps_ps.tile([128, 256], F32, tag="fps", name="svar")
            for fo in range(8):
                xsq = xsq_pool.tile([128, 256], F32R, tag="xsq", name="xsq")
                nc.gpsimd.tensor_mul(xsq[:], x_t[:, fo, :], x_t[:, fo, :])
                nc.tensor.matmul(smu[:], ones_f[:], x_t[:, fo, :],
                                 start=(fo == 0), stop=(fo == 7),
                                 skip_group_check=True)
                nc.tensor.matmul(svar[:], ones_f[:], xsq[:],
                                 start=(fo == 0), stop=(fo == 7),
                                 skip_group_check=True)
            mu_s = lnt_pool.tile([128, 256], F32, tag="mu", name="mu")
            nc.scalar.activation(mu_s[:], smu[:], AF.Copy, scale=1.0 / H)
            var = lnt_pool.tile([128, 256], F32, tag="var", name="var")
            nc.scalar.activation(var[:], svar[:], AF.Copy, scale=1.0 / H)
            mu2 = lnt_pool.tile([128, 256], F32, tag="mu2", name="mu2")
            nc.vector.tensor_mul(mu2[:], mu_s[:], mu_s[:])
            nc.vector.tensor_sub(var[:], var[:], mu2[:])
            rstd = lnt_pool.tile([128, 256], F32, tag="rst", name="rst")
            nc.scalar.activation(rstd[:], var[:], AF.Sqrt, bias=eps1c[:])
            nc.vector.reciprocal(rstd[:], rstd[:])
            for fo in range(8):
                t1 = lnt_pool.tile([128, 256], F32, tag="t1", name="t1")
                nc.vector.tensor_sub(t1[:], x_t[:, fo, :], mu_s[:])
                nc.vector.tensor_mul(h_t[:, fo, :], t1[:], rstd[:])
                nc.gpsimd.tensor_copy(h8_t[:, fo // 2, fo % 2, :],
                                      h_t[:, fo, :])
            _state["h"] = h_t
            # ---- FFN1: resident W1 (fp8 + fp8 residual), z = fps/32+b1 ----
            if P == 0:
                G_b = zg_pool.tile([128, 32, 256], BF16, tag="Gb", name="Gb")
                _state["G"] = G_b
            else:
                G_b = _state["G"]
            zch = None
            for ot in range(32):
                if ot % 4 == 0:
                    zch = zc_pool.tile([128, 4, 256], BF16, tag="zc",
                                       name="zc")
                fps = fps_ps.tile([128, 256], F32, tag="fps", name="f1")
                for kt in range(4):
                    nc.tensor.matmul(
                        fps[:], w18_sb[:, kt, :, 128 * ot:128 * (ot + 1)],
                        h8_t[:, kt, :, :], start=(kt == 0),
                        stop=False, perf_mode=DR)
                for kt in range(4):
                    nc.tensor.matmul(
                        fps[:], w1l_sb[:, kt, :, 128 * ot:128 * (ot + 1)],
                        h8_t[:, kt, :, :], start=False,
                        stop=(kt == 3), perf_mode=DR)
                with nc.allow_low_precision(reason="pre-gelu bf16"):
                    nc.vector.tensor_scalar(
                        out=zch[:, ot % 4, :], in0=fps[:],
                        scalar1=1.0 / SW,
                        scalar2=b1t_sb[:, ot:ot + 1],
                        op0=OP.mult, op1=OP.add)
                if ot % 4 == 3:
                    gi = ot // 4
                    nc.scalar.activation(
                        G_b[:, 4 * gi:4 * (gi + 1), :],
                        zch[:], AF.Gelu, scale=1.0)

        def ffn2_pass(P, fp):
            G_b = _state["G"]
            h_t = _state["h"]
            f2 = [f2_ps.tile([128, 256], F32, tag="f2", name="f2")
                  for _ in range(2)]
            for c4 in range(8):
                w2c = w2s_pool.tile([128, 4, 256], BF16, tag="w2c",
                                    name="w2c")
                nc.sync.dma_start(
                    w2c[:], w28_d[:, 4 * c4:4 * (c4 + 1),
                                  256 * fp:256 * (fp + 1)])
                for ktl in range(4):
                    for fh in range(2):
                        nc.tensor.matmul(
                            f2[fh][:],
                            w2c[:, ktl, 128 * fh:128 * (fh + 1)],
                            G_b[:, 4 * c4 + ktl, :],
                            start=(c4 == 0 and ktl == 0),
                            stop=(c4 == 7 and ktl == 3),
                            skip_group_check=True)
            # s = f2 + 32*h*g1 + 32*(be1+b2)  (f32, exact for host LN2)
            tt1 = tt1_pool.tile([128, 512], BF16, tag="tt", name="tt")
            s_t = sdr_pool.tile([128, 512], F32, tag="s", name="s")
            for fh in range(2):
                fo = 2 * fp + fh
                nc.vector.tensor_scalar(
                    out=tt1[:, 256 * fh:256 * (fh + 1)],
                    in0=h_t[:, fo, :],
                    scalar1=s1c_sb[:, fo:fo + 1],
                    scalar2=s2c_sb[:, fo:fo + 1],
                    op0=OP.mult, op1=OP.add)
                nc.vector.tensor_add(s_t[:, 256 * fh:256 * (fh + 1)],
                                     f2[fh][:],
                                     tt1[:, 256 * fh:256 * (fh + 1)])
            nc.sync.dma_start(
                out_d[:, 2 * fp:2 * (fp + 1), :]
                .rearrange("p a (c t) -> p c a t", c=2)[:, P],
                s_t[:].rearrange("p (a t) -> p a t", a=2))

        # ---------------- pipelined emission ----------------
        nc.sync.dma_start(wo8[:], wo8_d[:])
        # W1 prefetch (8MB) on the SP queue right after the repack: its
        # transfers fill the DMA-idle window during the w0/w1 exps and are
        # resident well before FFN1(P0)
        for c in range(4):
            nc.sync.dma_start(w18_sb[:, c], w18_d[:, c])
        for c in range(4):
            nc.sync.dma_start(w1l_sb[:, c], w1l_d[:, c])
        attn_window(0)
        attn_window(1)
        ln1_ffn1_gelu(0)
        attn_window(2)
        ffn2_pass(0, 0)
        ffn2_pass(0, 1)
        attn_window(3)
        ffn2_pass(0, 2)
        ffn2_pass(0, 3)
        ln1_ffn1_gelu(1)
        for fp in range(4):
            ffn2_pass(1, fp)

        for p in (xsq_pool, lnt_pool, tt1_pool, sdr_pool, w2s_pool,
                  zc_pool, zg_pool, hs_pool, xp_pool, xqt_pool, rbc_pool,
                  toff_pool, hp_pool, probs_pool, wo_pool, w1_pool,
                  att_pool, f2_ps, fps_ps, st_ps):
            p.release()

    nc.compile()
    return nc


_state = {}
_nc_cache = {}


def get_nc(key="full"):
    if key not in _nc_cache:
        _state.clear()
        _nc_cache[key] = build_program()
    return _nc_cache[key]


def host_prep(q, k, v, pad_mask, Wq, Wk, Wv, Wo, W1, b1, W2, b2,
              g1, be1, g2, be2):
    import ml_dtypes
    f = np.float32
    NPFP8 = ml_dtypes.float8_e4m3
    asf = lambda a: np.asarray(a, dtype=f)
    q, k, v = asf(q), asf(k), asf(v)
    pad = np.asarray(pad_mask)
    g1, be1, b1, b2 = asf(g1), asf(be1), asf(b1), asf(b2)

    def to8(a):
        return np.ascontiguousarray(a).astype(NPFP8)

    def wlay(Wmat, scale):  # [out, in] -> [128, in/256, 2, out] fp8
        wT = np.ascontiguousarray(asf(Wmat).T) * scale
        n_in, n_out = wT.shape
        return to8(wT.reshape(n_in // 256, 2, 128, n_out)
                   .transpose(2, 0, 1, 3))

    wq8 = wlay(Wq, SW)
    wk8 = wlay(Wk, SW)
    wv8 = wlay(Wv, SW)
    woT = np.ascontiguousarray(asf(Wo).T) * SW          # [in, out]
    wo8 = to8(woT.reshape(16, 64, H).transpose(1, 0, 2))
    W1p = asf(W1) * g1[None, :]                         # fold g1
    w18 = wlay(W1p, SW)
    w1res = (asf(W1p).T * SW) - \
        w18.transpose(1, 2, 0, 3).reshape(H, 4 * H).astype(f)
    w1l = to8(w1res.reshape(4, 2, 128, 4 * H).transpose(2, 0, 1, 3))
    import ml_dtypes as _mld
    w2T = np.ascontiguousarray(asf(W2).T) * SW
    w28 = np.ascontiguousarray(
        w2T.reshape(32, 128, H).transpose(1, 0, 2)).astype(_mld.bfloat16)
    b1p = b1 + asf(W1) @ be1                            # fold be1
    b1t = np.ascontiguousarray(b1p.reshape(32, 128).T)
    s1c = np.ascontiguousarray((SW * g1).reshape(8, 128).T)
    s2c = np.ascontiguousarray((SW * (be1 + b2)).reshape(8, 128).T)

    in_maps = []
    for c in range(NCORES):
        b_, s_ = c // GROUP, c % GROUP
        sl = slice(s_ * TQ, (s_ + 1) * TQ)
        pm = pad[b_].reshape(128, JB).astype(f)         # [t', j]
        kb = np.where(pm > 0, f(0.0), f(MASK_NEG)).astype(f)
        s1m = np.ascontiguousarray(pm * (EXPSC / 2))
        s2m = np.ascontiguousarray(pm)
        qm = np.ascontiguousarray(pm.T).reshape(-1)     # [128j + t']
        mrhs = np.zeros((128, 2, T), f)
        mrhs[0, 0, :] = 240.0 * (1.0 - qm)
        def xlay(x):
            xT = np.ascontiguousarray(x[b_, sl].T)      # [H, TQ]
            return to8(xT.reshape(4, 2, 128, TQ).transpose(2, 0, 1, 3))
        xqt = np.ascontiguousarray(
            (q[b_, sl].T * SQ).reshape(8, 128, TQ).transpose(1, 0, 2))
        in_maps.append(dict(
            xq8=xlay(q), xk8=xlay(k), xv8=xlay(v),
            xqt=np.ascontiguousarray(xqt, dtype=f),
            wq8=wq8, wk8=wk8, wv8=wv8, wo8=wo8, w18=w18, w1l=w1l,
            w28=w28,
            b1t=b1t, kb=np.ascontiguousarray(kb),
            s1m=s1m, s2m=s2m, mrhs=to8(mrhs), s1c=s1c, s2c=s2c,
        ))
    return in_maps


def kernel(q, k, v, pad_mask, Wq, Wk, Wv, Wo, W1, b1, W2, b2,
           g1, be1, g2, be2):
    from concourse.bass_utils import run_bass_kernel_spmd

    nc = get_nc()
    in_maps = host_prep(q, k, v, pad_mask, Wq, Wk, Wv, Wo, W1, b1, W2, b2,
                        g1, be1, g2, be2)
    res = run_bass_kernel_spmd(nc, in_maps, core_ids=list(range(NCORES)))
    g2f = np.asarray(g2, np.float32)
    be2f = np.asarray(be2, np.float32)
    out = np.empty((B, T, H), np.float32)
    eps2 = EPS * SW * SW
    for c in range(NCORES):
        b_, s_ = c // GROUP, c % GROUP
        s_fm = res.results[c]["out"]                    # [128, 8, TQ]
        s_tok = np.transpose(s_fm, (2, 1, 0)).reshape(TQ, H)
        mu = s_tok.mean(axis=1, keepdims=True)
        var = s_tok.var(axis=1, keepdims=True)
        o = (s_tok - mu) / np.sqrt(var + eps2)
        out[b_, s_ * TQ:(s_ + 1) * TQ, :] = \
            o * g2f[None, :] + be2f[None, :]
    return out


# revision 43
# speedup vs baseline: 1.0452x; 1.0092x over previous
"""Trainium2 Bass kernel for nn_MultiHeadAttention_51986284150861 (fp8 rev).

Full transformer block (MHA + LN1 + FFN; LN2 applied exactly on host) with
the reference's faithful torch-.view head split: head a attends over 2048
positions (j, t') where position (j, t') draws features 64j..64j+63 of
token 128a + t'.  Per core: batch b = c//4, 512 tokens = 4 head windows.
Zero collectives.

Pipelined emission so the ready-heap tile scheduler overlaps the ACT-bound
softmax of pair P+1 with the PE-bound FFN of pair P:
  attn(w0) attn(w1) | LN1(0) FFN1(0) gelu(0) | attn(w2) ~ FFN2(0) ~ attn(w3)
  | LN1(1) FFN1(1) gelu(1) FFN2(1)
PSUM rings: strips/wops [128,1024]x2 (4 banks), fps/stats [128,256]x2
(2 banks), f2/pav [128,512]x2 (2 banks).  FFN2 runs as 4 fo-pair passes
(W2 streamed once per pair).  FFN1 pre-gelu drains to SBUF (DVE/ACT
split), gelu batched as [128,1024] tiles to bound ACT table switches.
Softmax division: DVE recip -> Pool partition_broadcast -> DVE mul.
"""

import os
import sys

sys.path.insert(0, "/opt/trn_rl_repo")

from contextlib import ExitStack

import numpy as np

import concourse.bass as bass
import concourse.tile as tile
from concourse import bacc, mybir

F32 = mybir.dt.float32
F32R = mybir.dt.float32r
BF16 = mybir.dt.bfloat16
FP8 = mybir.dt.float8e4
DR = mybir.MatmulPerfMode.DoubleRow
AF = mybir.ActivationFunctionType
OP = mybir.AluOpType

B, T, H, NH, D = 2, 2048, 1024, 16, 64
EPS = 1e-5
NCORES = 8
GROUP = NCORES // B
TQ = T // GROUP              # 512 tokens/core
NW = 4                       # head windows per core
JB = 16                      # feature blocks
SW = 32.0                    # fp8 weight scale
SQ = 1024.0                  # q residual prescale (= SW*SW)
EXPSC = 1.0 / (8.0 * SW * SW)
MASK_NEG = -80.0
EPS1 = EPS * SQ * SQ         # LN1 eps (x is 1024-scaled)

# (w, jk, half) triples whose softmax goes DVE-affine + Pool-square instead
# of ACT exp.
N_OFF = int(os.environ.get("MHA_N_OFF", "0"))
OFF_JKS = [1, 3, 5, 7, 9, 11, 2, 6, 10, 14]
OFF_WINS = int(os.environ.get("MHA_OFF_WINS", "0"))
OFFLOAD = {(w, jk, half) for w in range(OFF_WINS) for half in range(2)
           for jk in OFF_JKS[:N_OFF]}
# FFN1 pre-gelu drains: which ot indices drain via ACT Copy (rest on DVE)
ACT_Z = int(os.environ.get("MHA_ACT_Z", "0"))  # of every 4


def _offload(w, jk, half):
    return (w, jk, half) in OFFLOAD


def build_program():
    nc = bacc.Bacc("TRN2", target_bir_lowering=False, debug=False)

    def din(name, shape, dt=F32):
        return nc.dram_tensor(name, list(shape), dt, kind="ExternalInput").ap()

    # inputs (per-core host layouts)
    xq8_d = din("xq8", (128, 4, 2, TQ), FP8)
    xk8_d = din("xk8", (128, 4, 2, TQ), FP8)
    xv8_d = din("xv8", (128, 4, 2, TQ), FP8)
    xqt_d = din("xqt", (128, 8, TQ))           # 1024*q^T feat-major f32
    wq8_d = din("wq8", (128, 4, 2, H), FP8)
    wk8_d = din("wk8", (128, 4, 2, H), FP8)
    wv8_d = din("wv8", (128, 4, 2, H), FP8)
    wo8_d = din("wo8", (64, 16, H), FP8)
    w18_d = din("w18", (128, 4, 2, 4 * H), FP8)
    w1l_d = din("w1l", (128, 4, 2, 4 * H), FP8)
    w28_d = din("w28", (128, 32, H), BF16)
    b1t_d = din("b1t", (128, 32))
    kb_d = din("kb", (128, JB))                # 0 / MASK_NEG per (t', j)
    s1m_d = din("s1m", (128, JB))              # mask*EXPSC/2 (offload path)
    s2m_d = din("s2m", (128, JB))              # mask (offload path)
    mrhs_d = din("mrhs", (128, 2, T), FP8)     # row0 pair0: 240*(1-qmask)
    s1c_d = din("s1c", (128, 8))               # 32*g1 per-feature cols
    s2c_d = din("s2c", (128, 8))               # 32*(be1+b2) cols
    # s = 32*(h + ff) feat-major; host does LN2 exactly
    out_d = nc.dram_tensor("out", [128, 8, TQ], BF16,
                           kind="ExternalOutput").ap()

    def mmr(ap):
        return ap.bitcast(F32R)

    with tile.TileContext(nc) as tc, ExitStack() as top:
        # ---------------- persistent pools ----------------
        consts = top.enter_context(tc.tile_pool(name="consts", bufs=1))
        kb_sb = consts.tile([128, JB], F32, tag="kb", name="kb")
        s1m_sb = consts.tile([128, JB], F32, tag="s1m", name="s1m")
        s2m_sb = consts.tile([128, JB], F32, tag="s2m", name="s2m")
        b1t_sb = consts.tile([128, 32], F32, tag="b1t", name="b1t")
        s1c_sb = consts.tile([128, 8], F32, tag="s1c", name="s1c")
        s2c_sb = consts.tile([128, 8], F32, tag="s2c", name="s2c")
        mrhs_sb = consts.tile([128, 2, T], FP8, tag="mrhs", name="mrhs")
        mz_sb = consts.tile([128, 2, 96], FP8, tag="mz", name="mz")
        ones_f = consts.tile([128, 128], F32R, tag="ones", name="ones")
        eps1c = consts.tile([128, 1], F32, tag="e1", name="e1")

        # PSUM rings
        st_ps = tc.alloc_tile_pool(name="stps", bufs=2, space="PSUM")
        fps_ps = tc.alloc_tile_pool(name="fpsps", bufs=2, space="PSUM")
        f2_ps = tc.alloc_tile_pool(name="f2ps", bufs=2, space="PSUM")

        # attention-persistent SBUF
        att_pool = tc.alloc_tile_pool(name="att", bufs=1)
        v8 = [att_pool.tile([128, JB, 96], FP8, tag=f"v8{w}", name="v8")
              for w in range(NW)]
        Qp = att_pool.tile([32, 2, 8192], FP8, tag="Qp", name="Qp")
        Kp = att_pool.tile([32, 2, 8192], FP8, tag="Kp", name="Kp")
        w1_pool = tc.alloc_tile_pool(name="w1p", bufs=1)
        w18_sb = w1_pool.tile([128, 4, 2, 4 * H], FP8, tag="w18", name="w18")
        w1l_sb = w1_pool.tile([128, 4, 2, 4 * H], FP8, tag="w1l", name="w1l")
        wo_pool = tc.alloc_tile_pool(name="wop", bufs=1)
        wo8 = wo_pool.tile([64, 16, H], FP8, tag="wo8", name="wo8")

        # ---------------- input DMAs: critical path first ----------------
        qtkt_pool = tc.alloc_tile_pool(name="qtkt", bufs=1)
        # free layout (w, jh, t) so the repack DMA APs balance to 3 dims
        qT = qtkt_pool.tile([128, 4, 8, 128], FP8, tag="qT", name="qT")
        kT = qtkt_pool.tile([128, 4, 8, 128], FP8, tag="kT", name="kT")

        with tc.tile_pool(name="wproj", bufs=1) as wproj, \
             tc.tile_pool(name="xin", bufs=1) as xin:
            wq8 = wproj.tile([128, 4, 2, H], FP8, tag="wq", name="wq")
            xq8 = xin.tile([128, 4, 2, TQ], FP8, tag="xq", name="xq")
            wk8 = wproj.tile([128, 4, 2, H], FP8, tag="wk", name="wk")
            xk8 = xin.tile([128, 4, 2, TQ], FP8, tag="xk", name="xk")
            wv8 = wproj.tile([128, 4, 2, H], FP8, tag="wv", name="wv")
            xv8 = xin.tile([128, 4, 2, TQ], FP8, tag="xv", name="xv")
            # SP HWDGE queue: critical QKV inputs
            nc.sync.dma_start(wq8[:], wq8_d[:])
            nc.sync.dma_start(xq8[:], xq8_d[:])
            nc.sync.dma_start(wk8[:], wk8_d[:])
            nc.sync.dma_start(xk8[:], xk8_d[:])
            nc.sync.dma_start(wv8[:], wv8_d[:])
            nc.sync.dma_start(xv8[:], xv8_d[:])
            # ACT HWDGE queue: the flag rhs (needed by the first AV)
            nc.scalar.dma_start(mrhs_sb[:], mrhs_d[:])
            # SWDGE (Pool seq): consts off critical path
            nc.gpsimd.dma_start(kb_sb[:], kb_d[:])
            nc.gpsimd.dma_start(s1m_sb[:], s1m_d[:])
            nc.gpsimd.dma_start(s2m_sb[:], s2m_d[:])
            nc.gpsimd.dma_start(b1t_sb[:], b1t_d[:])
            nc.gpsimd.dma_start(s1c_sb[:], s1c_d[:])
            nc.gpsimd.dma_start(s2c_sb[:], s2c_d[:])
            nc.gpsimd.memset(mz_sb[:], 0.0)
            nc.gpsimd.memset(mz_sb[0:1, 0:1, 64:65], 240.0)
            nc.vector.memset(ones_f[:].bitcast(F32), 1.0)
            nc.vector.memset(eps1c[:], EPS1)
            for w in range(NW):
                nc.gpsimd.memset(v8[w][:, :, 64:65], 1.0)

            # ============ P1: projections (fp8 DoubleRow) ============
            for (wt, xt, dst) in ((wq8, xq8, qT), (wk8, xk8, kT)):
                for ht in range(8):
                    ps = f2_ps.tile([128, TQ], F32, tag="f2", name="pp")
                    for kt in range(4):
                        nc.tensor.matmul(
                            ps[:], wt[:, kt, :, 128 * ht:128 * (ht + 1)],
                            xt[:, kt], start=(kt == 0), stop=(kt == 3),
                            perf_mode=DR)
                    nc.vector.tensor_copy(
                        dst[:, :, ht, :],
                        ps[:].rearrange("p (w t) -> p w t", t=128))
            for tt in range(NW):
                for nk in range(2):
                    ps = f2_ps.tile([128, TQ], F32, tag="f2", name="pp")
                    for kt in range(4):
                        nc.tensor.matmul(
                            ps[:], xv8[:, kt, :, 128 * tt:128 * (tt + 1)],
                            wv8[:, kt, :, 512 * nk:512 * (nk + 1)],
                            start=(kt == 0), stop=(kt == 3), perf_mode=DR)
                    nc.vector.tensor_copy(
                        v8[tt][:, 8 * nk:8 * (nk + 1), 0:64],
                        ps[:].rearrange("p (a b) -> p a b", b=64))

        # ============ P1.5: repack Q/K -> [32, 2, (w, j, t')] ============
        # 8 consolidated DMAs (one per src x i x hh), covering all windows.
        for (src, dst) in ((qT, Qp), (kT, Kp)):
            for i in range(2):
                for hh in range(2):
                    s_ap = src[32 * i + 64 * hh:32 * i + 64 * hh + 32]
                    d_ap = dst[:, i] \
                        .rearrange("p (w jh two t) -> p w jh two t",
                                   w=4, jh=8, two=2, t=128)[:, :, :, hh, :]
                    nc.sync.dma_start(d_ap, s_ap)
        qtkt_pool.release()

        # pools created after the P1 input pools release their SBUF
        probs_pool = tc.alloc_tile_pool(name="probs", bufs=8)
        hp_pool = tc.alloc_tile_pool(name="hp", bufs=2)
        toff_pool = tc.alloc_tile_pool(name="toff", bufs=2)
        rbc_pool = tc.alloc_tile_pool(name="rbc", bufs=2)
        xqt_pool = tc.alloc_tile_pool(name="xqtp", bufs=1)
        xp_pool = tc.alloc_tile_pool(name="xp", bufs=1)
        hs_pool = tc.alloc_tile_pool(name="hs", bufs=1)
        zg_pool = tc.alloc_tile_pool(name="zg", bufs=1)
        zc_pool = tc.alloc_tile_pool(name="zc", bufs=2)
        w2s_pool = tc.alloc_tile_pool(name="w2s", bufs=5)
        sdr_pool = tc.alloc_tile_pool(name="sdr", bufs=2)
        tt1_pool = tc.alloc_tile_pool(name="tt1", bufs=2)
        lnt_pool = tc.alloc_tile_pool(name="lnt", bufs=1)
        xsq_pool = tc.alloc_tile_pool(name="xsq", bufs=1)

        # ---------------- stage emitters ----------------
        def attn_window(w):
            P_ = w // 2
            wh = w % 2
            if wh == 0:
                xqt_t = xqt_pool.tile([128, 8, 256], F32, tag="xqt",
                                      name="xqt")
                nc.scalar.dma_start(xqt_t[:],
                                    xqt_d[:, :, 256 * P_:256 * (P_ + 1)])
                x_t = xp_pool.tile([128, 8, 256], F32R, tag="x", name="x")
                _state["xqt"] = xqt_t
                _state["x"] = x_t
            else:
                xqt_t = _state["xqt"]
                x_t = _state["x"]
            hp = hp_pool.tile([64, T], FP8, tag="hp", name="hp")
            for half in range(2):
                pairs = []
                for jk in range(JB):
                    if jk % 2 == 0:
                        pairs.append(probs_pool.tile([128, 2, 1024], FP8,
                                                     tag="pr", name="pr"))
                    pr = pairs[jk // 2][:, jk % 2, :]
                    strip = st_ps.tile([128, 1024], F32, tag="st",
                                       name="st")
                    for qc in range(2):
                        qoff = 2048 * w + 1024 * half + 512 * qc
                        nc.tensor.matmul(
                            strip[:, 512 * qc:512 * (qc + 1)],
                            Kp[:, :, 2048 * w + 128 * jk:
                               2048 * w + 128 * (jk + 1)],
                            Qp[:, :, qoff:qoff + 512],
                            start=True, stop=True, perf_mode=DR,
                            skip_group_check=True)
                    if _offload(w, jk, half):
                        t_sb = toff_pool.tile([128, 1024], BF16, tag="t",
                                              name="t")
                        nc.vector.tensor_scalar(
                            out=t_sb[:], in0=strip[:],
                            scalar1=s1m_sb[:, jk:jk + 1],
                            scalar2=s2m_sb[:, jk:jk + 1],
                            op0=OP.mult, op1=OP.add)
                        nc.gpsimd.tensor_mul(pr, t_sb[:], t_sb[:])
                    else:
                        nc.scalar.activation(pr, strip[:], AF.Exp,
                                             bias=kb_sb[:, jk:jk + 1],
                                             scale=EXPSC)
                pavs = [fps_ps.tile([128, 512], F32, tag="fps", name="pav")
                        for _ in range(2)]
                for m in range(8):
                    for qc in range(2):
                        sl = slice(512 * qc, 512 * (qc + 1))
                        nc.tensor.matmul(
                            pavs[qc][0:65, :],
                            v8[w][:, 2 * m:2 * m + 2, 0:65],
                            pairs[m][:, :, sl],
                            start=(m == 0), stop=False, perf_mode=DR,
                            skip_group_check=True)
                for qc in range(2):
                    moff = 1024 * half + 512 * qc
                    nc.tensor.matmul(
                        pavs[qc][0:65, :], mz_sb[:, :, 0:65],
                        mrhs_sb[:, :, moff:moff + 512],
                        start=False, stop=True, perf_mode=DR,
                        skip_group_check=True)
                    # division: recip row -> Pool broadcast -> DVE mul
                    rbc = rbc_pool.tile([64, 512], BF16, tag="rbc",
                                        name="rbc")
                    with nc.allow_low_precision(reason="softmax recip bf16"):
                        nc.vector.reciprocal(rbc[0:1, :], pavs[qc][64:65, :])
                    nc.gpsimd.partition_broadcast(rbc[:], rbc[0:1, :])
                    nc.vector.tensor_mul(
                        hp[:, 1024 * half + 512 * qc:
                           1024 * half + 512 * (qc + 1)],
                        pavs[qc][0:64, :], rbc[:])
            # ---- Wo(w) + residual ----
            wops = st_ps.tile([128, 1024], F32, tag="st", name="wo")
            for fo in range(8):
                for m in range(8):
                    nc.tensor.matmul(
                        wops[:, 128 * fo:128 * (fo + 1)],
                        wo8[:, 2 * m:2 * m + 2, 128 * fo:128 * (fo + 1)],
                        hp[:].rearrange("p (j t) -> p j t", t=128)
                        [:, 2 * m:2 * m + 2, :],
                        start=(m == 0), stop=(m == 7), perf_mode=DR,
                        skip_group_check=True)
            nc.vector.tensor_add(
                x_t[:, :, 128 * wh:128 * (wh + 1)],
                wops[:].rearrange("p (a b) -> p a b", b=128),
                xqt_t[:, :, 128 * wh:128 * (wh + 1)])

        def ln1_ffn1_gelu(P):
            x_t = _state["x"]
            h_t = hs_pool.tile([128, 8, 256], BF16, tag="h", name="h")
            h8_t = hs_pool.tile([128, 4, 2, 256], FP8, tag="h8", name="h8")
            # ---- LN1 stats (fp32r ones-matmuls) ----
            smu = fps_ps.tile([128, 256], F32, tag="fps", name="smu")
            svar = fps_ps.tile([128, 256], F32, tag="fps", name="svar")
            for fo in range(8):
                xsq = xsq_pool.tile([128, 256], F32R, tag="xsq", name="xsq")
                nc.gpsimd.tensor_mul(xsq[:], x_t[:, fo, :], x_t[:, fo, :])
                nc.tensor.matmul(smu[:], ones_f[:], x_t[:, fo, :],
                                 start=(fo == 0), stop=(fo == 7),
                                 skip_group_check=True)
                nc.tensor.matmul(svar[:], ones_f[:], xsq[:],
                                 start=(fo == 0), stop=(fo == 7),
                                 skip_group_check=True)
            mu_s = lnt_pool.tile([128, 256], F32, tag="mu", name="mu")
            nc.scalar.activation(mu_s[:], smu[:], AF.Copy, scale=1.0 / H)
            var = lnt_pool.tile([128, 256], F32, tag="var", name="var")
            nc.scalar.activation(var[:], svar[:], AF.Copy, scale=1.0 / H)
            mu2 = lnt_pool.tile([128, 256], F32, tag="t1", name="mu2")
            nc.vector.tensor_mul(mu2[:], mu_s[:], mu_s[:])
            nc.vector.tensor_sub(var[:], var[:], mu2[:])
            rstd = lnt_pool.tile([128, 256], F32, tag="rst", name="rst")
            nc.scalar.activation(rstd[:], var[:], AF.Sqrt, bias=eps1c[:])
            nc.vector.reciprocal(rstd[:], rstd[:])
            for fo in range(8):
                t1 = lnt_pool.tile([128, 256], F32, tag="t1", name="t1")
                nc.vector.tensor_sub(t1[:], x_t[:, fo, :], mu_s[:])
                nc.vector.tensor_mul(h_t[:, fo, :], t1[:], rstd[:])
                nc.gpsimd.tensor_copy(h8_t[:, fo // 2, fo % 2, :],
                                      h_t[:, fo, :])
            _state["h"] = h_t
            # ---- FFN1: resident W1 (fp8 + fp8 residual), z = fps/32+b1 ----
            if P == 0:
                G_b = zg_pool.tile([128, 32, 256], BF16, tag="Gb", name="Gb")
                _state["G"] = G_b
            else:
                G_b = _state["G"]
            zch = None
            for ot in range(32):
                if ot % 4 == 0:
                    zch = zc_pool.tile([128, 4, 256], BF16, tag="zc",
                                       name="zc")
                fps = fps_ps.tile([128, 256], F32, tag="fps", name="f1")
                for kt in range(4):
                    nc.tensor.matmul(
                        fps[:], w18_sb[:, kt, :, 128 * ot:128 * (ot + 1)],
                        h8_t[:, kt, :, :], start=(kt == 0),
                        stop=False, perf_mode=DR)
                for kt in range(4):
                    nc.tensor.matmul(
                        fps[:], w1l_sb[:, kt, :, 128 * ot:128 * (ot + 1)],
                        h8_t[:, kt, :, :], start=False,
                        stop=(kt == 3), perf_mode=DR)
                with nc.allow_low_precision(reason="pre-gelu bf16"):
                    nc.vector.tensor_scalar(
                        out=zch[:, ot % 4, :], in0=fps[:],
                        scalar1=1.0 / SW,
                        scalar2=b1t_sb[:, ot:ot + 1],
                        op0=OP.mult, op1=OP.add)
                if ot % 4 == 3:
                    gi = ot // 4
                    nc.scalar.activation(
                        G_b[:, 4 * gi:4 * (gi + 1), :],
                        zch[:], AF.Gelu, scale=1.0)

        def ffn2_pass(P, fp):
            G_b = _state["G"]
            h_t = _state["h"]
            f2 = [f2_ps.tile([128, 256], F32, tag="f2", name="f2")
                  for _ in range(2)]
            for c4 in range(8):
                w2c = w2s_pool.tile([128, 4, 256], BF16, tag="w2c",
                                    name="w2c")
                nc.sync.dma_start(
                    w2c[:], w28_d[:, 4 * c4:4 * (c4 + 1),
                                  256 * fp:256 * (fp + 1)])
                for ktl in range(4):
                    for fh in range(2):
                        nc.tensor.matmul(
                            f2[fh][:],
                            w2c[:, ktl, 128 * fh:128 * (fh + 1)],
                            G_b[:, 4 * c4 + ktl, :],
                            start=(c4 == 0 and ktl == 0),
                            stop=(c4 == 7 and ktl == 3),
                            skip_group_check=True)
            # s = f2 + 32*h*g1 + 32*(be1+b2)  (f32, exact for host LN2)
            tt1 = tt1_pool.tile([128, 512], BF16, tag="tt", name="tt")
            s_t = sdr_pool.tile([128, 512], BF16, tag="s", name="s")
            for fh in range(2):
                fo = 2 * fp + fh
                nc.vector.tensor_scalar(
                    out=tt1[:, 256 * fh:256 * (fh + 1)],
                    in0=h_t[:, fo, :],
                    scalar1=s1c_sb[:, fo:fo + 1],
                    scalar2=s2c_sb[:, fo:fo + 1],
                    op0=OP.mult, op1=OP.add)
                with nc.allow_low_precision(reason="s bf16; host LN2"):
                    nc.vector.tensor_add(s_t[:, 256 * fh:256 * (fh + 1)],
                                         f2[fh][:],
                                         tt1[:, 256 * fh:256 * (fh + 1)])
            nc.sync.dma_start(
                out_d[:, 2 * fp:2 * (fp + 1), :]
                .rearrange("p a (c t) -> p c a t", c=2)[:, P],
                s_t[:].rearrange("p (a t) -> p a t", a=2))

        # ---------------- pipelined emission ----------------
        nc.sync.dma_start(wo8[:], wo8_d[:])
        # W1 prefetch (8MB) on the SP queue right after the repack: its
        # transfers fill the DMA-idle window during the w0/w1 exps and are
        # resident well before FFN1(P0)
        for c in range(4):
            nc.sync.dma_start(w18_sb[:, c], w18_d[:, c])
        for c in range(4):
            nc.sync.dma_start(w1l_sb[:, c], w1l_d[:, c])
        attn_window(0)
        attn_window(1)
        ln1_ffn1_gelu(0)
        attn_window(2)
        ffn2_pass(0, 0)
        ffn2_pass(0, 1)
        attn_window(3)
        ffn2_pass(0, 2)
        ffn2_pass(0, 3)
        ln1_ffn1_gelu(1)
        for fp in range(4):
            ffn2_pass(1, fp)

        for p in (xsq_pool, lnt_pool, tt1_pool, sdr_pool, w2s_pool,
                  zc_pool, zg_pool, hs_pool, xp_pool, xqt_pool, rbc_pool,
                  toff_pool, hp_pool, probs_pool, wo_pool, w1_pool,
                  att_pool, f2_ps, fps_ps, st_ps):
            p.release()

    nc.compile()
    return nc


_state = {}
_nc_cache = {}


def get_nc(key="full"):
    if key not in _nc_cache:
        _state.clear()
        _nc_cache[key] = build_program()
    return _nc_cache[key]


def host_prep(q, k, v, pad_mask, Wq, Wk, Wv, Wo, W1, b1, W2, b2,
              g1, be1, g2, be2):
    import ml_dtypes
    f = np.float32
    NPFP8 = ml_dtypes.float8_e4m3
    asf = lambda a: np.asarray(a, dtype=f)
    q, k, v = asf(q), asf(k), asf(v)
    pad = np.asarray(pad_mask)
    g1, be1, b1, b2 = asf(g1), asf(be1), asf(b1), asf(b2)

    def to8(a):
        return np.ascontiguousarray(a).astype(NPFP8)

    def wlay(Wmat, scale):  # [out, in] -> [128, in/256, 2, out] fp8
        wT = np.ascontiguousarray(asf(Wmat).T) * scale
        n_in, n_out = wT.shape
        return to8(wT.reshape(n_in // 256, 2, 128, n_out)
                   .transpose(2, 0, 1, 3))

    wq8 = wlay(Wq, SW)
    wk8 = wlay(Wk, SW)
    wv8 = wlay(Wv, SW)
    woT = np.ascontiguousarray(asf(Wo).T) * SW          # [in, out]
    wo8 = to8(woT.reshape(16, 64, H).transpose(1, 0, 2))
    W1p = asf(W1) * g1[None, :]                         # fold g1
    w18 = wlay(W1p, SW)
    w1res = (asf(W1p).T * SW) - \
        w18.transpose(1, 2, 0, 3).reshape(H, 4 * H).astype(f)
    w1l = to8(w1res.reshape(4, 2, 128, 4 * H).transpose(2, 0, 1, 3))
    import ml_dtypes as _mld
    w2T = np.ascontiguousarray(asf(W2).T) * SW
    w28 = np.ascontiguousarray(
        w2T.reshape(32, 128, H).transpose(1, 0, 2)).astype(_mld.bfloat16)
    b1p = b1 + asf(W1) @ be1                            # fold be1
    b1t = np.ascontiguousarray(b1p.reshape(32, 128).T)
    s1c = np.ascontiguousarray((SW * g1).reshape(8, 128).T)
    s2c = np.ascontiguousarray((SW * (be1 + b2)).reshape(8, 128).T)

    in_maps = []
    for c in range(NCORES):
        b_, s_ = c // GROUP, c % GROUP
        sl = slice(s_ * TQ, (s_ + 1) * TQ)
        pm = pad[b_].reshape(128, JB).astype(f)         # [t', j]
        kb = np.where(pm > 0, f(0.0), f(MASK_NEG)).astype(f)
        s1m = np.ascontiguousarray(pm * (EXPSC / 2))
        s2m = np.ascontiguousarray(pm)
        qm = np.ascontiguousarray(pm.T).reshape(-1)     # [128j + t']
        mrhs = np.zeros((128, 2, T), f)
        mrhs[0, 0, :] = 240.0 * (1.0 - qm)
        def xlay(x):
            xT = np.ascontiguousarray(x[b_, sl].T)      # [H, TQ]
            return to8(xT.reshape(4, 2, 128, TQ).transpose(2, 0, 1, 3))
        xqt = np.ascontiguousarray(
            (q[b_, sl].T * SQ).reshape(8, 128, TQ).transpose(1, 0, 2))
        in_maps.append(dict(
            xq8=xlay(q), xk8=xlay(k), xv8=xlay(v),
            xqt=np.ascontiguousarray(xqt, dtype=f),
            wq8=wq8, wk8=wk8, wv8=wv8, wo8=wo8, w18=w18, w1l=w1l,
            w28=w28,
            b1t=b1t, kb=np.ascontiguousarray(kb),
            s1m=s1m, s2m=s2m, mrhs=to8(mrhs), s1c=s1c, s2c=s2c,
        ))
    return in_maps


def kernel(q, k, v, pad_mask, Wq, Wk, Wv, Wo, W1, b1, W2, b2,
           g1, be1, g2, be2):
    from concourse.bass_utils import run_bass_kernel_spmd

    nc = get_nc()
    in_maps = host_prep(q, k, v, pad_mask, Wq, Wk, Wv, Wo, W1, b1, W2, b2,
                        g1, be1, g2, be2)
    res = run_bass_kernel_spmd(nc, in_maps, core_ids=list(range(NCORES)))
    g2f = np.asarray(g2, np.float32)
    be2f = np.asarray(be2, np.float32)
    out = np.empty((B, T, H), np.float32)
    eps2 = EPS * SW * SW
    for c in range(NCORES):
        b_, s_ = c // GROUP, c % GROUP
        s_fm = np.asarray(res.results[c]["out"], np.float32)  # [128,8,TQ]
        s_tok = np.transpose(s_fm, (2, 1, 0)).reshape(TQ, H)
        mu = s_tok.mean(axis=1, keepdims=True)
        var = s_tok.var(axis=1, keepdims=True)
        o = (s_tok - mu) / np.sqrt(var + eps2)
        out[b_, s_ * TQ:(s_ + 1) * TQ, :] = \
            o * g2f[None, :] + be2f[None, :]
    return out
